# revision 89
# baseline (speedup 1.0000x reference)
"""Self-contained Trainium2 Bass kernel for causal attention with relative
position bias (B=4, T=1024, D=1024, H=16, dh=64), SPMD across 8 NeuronCores.

Sharding: core = (batch b = core//2, head-half g = core%2). Each core computes
QKV projections for its 8 heads, causal attention, and a partial output
projection; partials are summed pairwise with on-device ReduceScatters (one
per T/4 output quarter, written directly into output parameters).

Schedule: QKV projection groups are interleaved with per-head logits/exp/AV so
the Scalar engine's exp work overlaps the projection matmuls. Softmax
normalization: for early chunks (C0-C5, while ps_mm is busy with QKV) the
reciprocal row is columnarized through a DRAM bounce and broadcast back by
DMA; for later chunks a DVE row-reciprocal (in two halves) is broadcast
across 64 partitions by a tiny ones-stationary PE matmul into the free ps_mm
bank, then copied to SBUF and multiplied in. Drain ops are deferred in each
engine queue (with dedicated semaphores) so their cross-engine waits never
block QKV copies or the PE stream. Outputs leave via three ReduceScatters
(t rows [0:256), [256:512), [512:1024)) with a warm-up dummy collective to
absorb the CC cold start; bo is added on the host after gathering.

Layouts (per core):
  xT    [128, 8, 1024]  bf16   x[b].T as [d%128, d//128, t]
  wq/wk [128, 8, 512]   bf16   W[:, g*512:+512] as [d%128, d//128, n]
  wv    [128, 8, 512]   bf16   same
  wo    [128, 4, 1024]  bf16   Wo[g*512:+512, :] as [n%128, n//128, m]
  dbias [128, 8, 1024]  bf16   per local head: bias[j%128, i-128jb] with
                               causal mask folded in as -1e9
  QT/KT [128, 4, 1024]  bf16   [n%128, n//128, t]  (channel-major)
  V_aug [128, 8, 8, 65] bf16   [t%128, t//128, h, c] with ones column c=64
  pt    [128, 2, 8, 1024] bf16 exp((logits+bias)/64), [j%128, jb, i]
  attT  [128, 4, 1024]  bf16   normalized attention output, channel-major
"""
import sys

sys.path.insert(0, "/opt/trn_rl_repo")

import numpy as np
import ml_dtypes

B, T, D = 4, 1024, 1024
H, DH = 16, 64
HL, NL = 8, 512  # local heads / channels per core
NCORES = 8
NEG = -1.0e9

_CACHE = {}


def _phase_tiles(p):
    """Logit tiles (jb, i0, w) for query phase p (cols 512p..512p+512)."""
    tiles = []
    if p == 0:
        for jb in range(4):
            tiles.append((jb, 128 * jb, 512 - 128 * jb))
    else:
        for jb in range(8):
            i0 = max(512, 128 * jb)
            tiles.append((jb, i0, 1024 - i0))
    return tiles


def _build():
    from concourse import bass
    from contextlib import ExitStack

    mybir = bass.mybir
    f32, bf16 = mybir.dt.float32, mybir.dt.bfloat16

    nc = bass.Bass(target_bir_lowering=False, debug=False)
    xT = nc.declare_dram_parameter("xT", [128, 8, T], bf16, isOutput=False)
    wq0 = nc.declare_dram_parameter("wq0", [128, 8, 128], bf16, isOutput=False)
    wq1 = nc.declare_dram_parameter("wq1", [128, 8, 384], bf16, isOutput=False)
    wk0 = nc.declare_dram_parameter("wk0", [128, 8, 128], bf16, isOutput=False)
    wk1 = nc.declare_dram_parameter("wk1", [128, 8, 384], bf16, isOutput=False)
    wv = nc.declare_dram_parameter("wv", [128, 8, NL], bf16, isOutput=False)
    wo = nc.declare_dram_parameter("wo", [128, 4, D], bf16, isOutput=False)
    dbias = nc.declare_dram_parameter("dbias", [128, HL, T], bf16, isOutput=False)
    ident = nc.declare_dram_parameter("ident", [128, 128], bf16, isOutput=False)
    Q_TBS = [[0, 1], [2, 3], [4, 5], [6, 7]]  # t-blocks per output chunk
    outs_p = [nc.declare_dram_parameter(f"out{q}", [64 * len(t), D], bf16, isOutput=True)
              for q, t in enumerate(Q_TBS)]

    partials = [nc.dram_tensor(f"partial{q}", [128 * len(t), D], bf16)
                for q, t in enumerate(Q_TBS)]
    reds = [nc.dram_tensor(f"red{q}", [64 * len(t), D], bf16)
            for q, t in enumerate(Q_TBS)]
    warm_in = nc.dram_tensor("warm_in", [2, 64], bf16)
    warm_out = nc.dram_tensor("warm_out", [1, 64], bf16)
    r2_dram = nc.dram_tensor("r2_dram", [2, 128, 4], f32)

    ctx = ExitStack()
    sem = lambda n: ctx.enter_context(nc.semaphore(n))
    sb = lambda n, shape, dt: ctx.enter_context(nc.sbuf_tensor(n, shape, dt))
    ps = lambda n, shape: ctx.enter_context(nc.psum_tensor(n, shape, f32))

    s_xT = sem("s_xT")
    s_wq = sem("s_wq")
    s_wk = sem("s_wk")
    s_wv = sem("s_wv")
    s_wo = sem("s_wo")
    s_d = sem("s_d")
    s_pe = sem("s_pe")
    s_dve = sem("s_dve")
    s_dr = sem("s_dr")
    s_act = sem("s_act")
    s_rd = [sem("s_rd0"), sem("s_rd1")]
    s_r2 = [sem("s_r20"), sem("s_r21")]
    s_rbc = [sem("s_rbc0"), sem("s_rbc1")]
    s_tm = [sem("s_tm0"), sem("s_tm1")]
    s_out = sem("s_out")
    s_cc = sem("s_cc")
    s_fin = sem("s_fin")
    s_bc = sem("s_bc")
    s_rp = sem("s_rp")
    s_bm = sem("s_bm")

    xT_sb = sb("xT_sb", [128, 8, T], bf16)
    wq_sb = sb("wq_sb", [128, 8, NL], bf16)
    wk_sb = sb("wk_sb", [128, 8, NL], bf16)
    wv_sb = sb("wv_sb", [128, 8, NL], bf16)
    wo_sb = sb("wo_sb", [128, 4, D], bf16)
    qt_sb = sb("qt_sb", [128, 4, T], bf16)
    kt_sb = sb("kt_sb", [128, 4, T], bf16)
    va_sb = sb("va_sb", [128, 8, HL, 65], bf16)
    pt_sb = sb("pt_sb", [128, 4, 8, T], bf16)  # ph0: slot h%4, ph1: slot h%2
    at_sb = sb("at_sb", [128, 4, T], bf16)
    db_all = sb("db_all", [128, HL, T], bf16)
    id_sb = sb("id_sb", [128, 128], bf16)
    srow = sb("srow", [65, 2, 512], f32)  # recip rows live on partition 64
    sP = sb("sP", [128, 2, 4], f32)
    on_sb = sb("on_sb", [65, 64], f32)    # ones row for the PE broadcast
    rbc = [sb("rbc0", [64, 512], f32), sb("rbc1", [64, 512], f32)]
    tmp = [sb("tmp0", [64, 512], bf16), sb("tmp1", [64, 512], bf16)]
    stg = sb("stg", [128, 8, 512], bf16)

    ps_mm = [ps("ps_mm0", [128, 512]), ps("ps_mm1", [128, 512])]
    ps_lg = [ps("ps_lg0", [128, 512]), ps("ps_lg1", [128, 512]), ps("ps_lg2", [128, 512])]
    ps_at = [ps("ps_at0", [65, 512]), ps("ps_at1", [65, 512]), ps("ps_at2", [65, 512])]

    # ---- plan ----
    ops = {k: [] for k in ("sp", "pe", "dve", "act", "gp")}

    def wait(eng, s, v):
        ops[eng].append(("wait", s, v))

    def op(eng, fn, inc=None):
        ops[eng].append(("op", fn, inc))

    cnt = {"pe": 0, "dve": 0, "act": 0, "dr": 0, "rp": 0, "bm": 0, "out": 0, "bc": 0,
           "rd": [0, 0], "r2": [0, 0], "rbc": [0, 0], "tm": [0, 0]}
    tmch = {0: [], 1: []}  # chunks with tm DMAs per phase
    rec = {}

    # --- input DMAs: critical loads split across both HWDGE queues ---
    op("sp", lambda e: e.dma_start(out=xT_sb[:, :, 0:256], in_=xT[:, :, 0:256]), (s_xT, 16))
    op("sp", lambda e: e.dma_start(out=xT_sb[:, :, 256:512], in_=xT[:, :, 256:512]), (s_xT, 16))
    op("sp", lambda e: e.dma_start(out=db_all[:, 0:4, :], in_=dbias[:, 0:4, :]), (s_d, 16))
    op("sp", lambda e: e.dma_start(out=id_sb[:], in_=ident[:, :]), (s_d, 16))
    op("sp", lambda e: e.dma_start(out=wv_sb[:], in_=wv[:]), (s_wv, 16))
    op("sp", lambda e: e.dma_start(out=db_all[:, 4:8, :], in_=dbias[:, 4:8, :]), (s_d, 16))
    op("sp", lambda e: e.dma_start(out=xT_sb[:, :, 512:1024], in_=xT[:, :, 512:1024]), (s_xT, 16))
    op("sp", lambda e: e.dma_start(out=wo_sb[:], in_=wo[:]), (s_wo, 16))

    op("act", lambda e: e.dma_start(out=wq_sb[:, :, 0:128], in_=wq0[:]), (s_wq, 16))
    op("act", lambda e: e.dma_start(out=wk_sb[:, :, 0:128], in_=wk0[:]), (s_wk, 16))
    op("act", lambda e: e.dma_start(out=wq_sb[:, :, 128:512], in_=wq1[:]), (s_wq, 16))
    op("act", lambda e: e.dma_start(out=wk_sb[:, :, 128:512], in_=wk1[:]), (s_wk, 16))

    # cold-start the collective engine with a tiny dummy ReduceScatter
    op("gp", lambda e: e.collective_compute(
        "ReduceScatter", bass.mybir.AluOpType.add,
        replica_groups=[[0, 1], [2, 3], [4, 5], [6, 7]],
        ins=[warm_in.ap().opt()], outs=[warm_out.ap().opt()]), (s_cc, 1))

    # --- DVE memsets: pt invalid regions + V ones column ---
    for sl in range(4):          # ph0 (cols 0:512): jb 1-3 need leading zeros
        for jb in range(1, 4):
            op("dve", (lambda sl=sl, j=jb: lambda e: e.memset(pt_sb[:, sl, j, 0:128 * j], 0.0))(), (s_dve, 1))
            cnt["dve"] += 1
    for sl in range(4):          # ph1 (cols 512:1024): jb 5-7 need leading zeros
        for jb in range(5, 8):
            op("dve", (lambda sl=sl, j=jb: lambda e: e.memset(pt_sb[:, sl, j, 512:128 * j], 0.0))(), (s_dve, 1))
            cnt["dve"] += 1
    op("dve", lambda e: e.memset(va_sb[:, :, :, 64:65], 1.0), (s_dve, 1))
    cnt["dve"] += 1
    op("dve", lambda e: e.memset(on_sb[64:65, :], 1.0), (s_dve, 1))
    cnt["dve"] += 1

    # HAM warm-up: dummy matmuls on untouched SBUF during the input-DMA wait.
    for _w in range(14):
        op("pe", (lambda: lambda e: e.matmul(
            ps_lg[0][:, :], stg[:, 0, 0:128], stg[:, 1, :],
            start=True, stop=True))(), None)

    # --- QKV projection groups ---
    def plan_qkv(item, gidx):
        kind = item[0]
        slot = ps_mm[gidx % 2]
        if gidx >= 2:
            wait("pe", s_dve, rec[("copy", gidx - 2)])
        for db in range(8):
            st, sp_ = db == 0, db == 7
            if kind in ("q", "k"):
                _, nb, tc = item
                w = wq_sb if kind == "q" else wk_sb
                fn = (lambda w=w, nb=nb, tc=tc, db=db, slot=slot, st=st, sp_=sp_: lambda e: e.matmul(
                    slot[:, :], w[:, db, nb * 128:(nb + 1) * 128], xT_sb[:, db, tc * 512:(tc + 1) * 512],
                    start=st, stop=sp_))()
            else:
                _, tb = item
                fn = (lambda tb=tb, db=db, slot=slot, st=st, sp_=sp_: lambda e: e.matmul(
                    slot[:, :], xT_sb[:, db, tb * 128:(tb + 1) * 128], wv_sb[:, db, 0:NL],
                    start=st, stop=sp_))()
            op("pe", fn, (s_pe, 1) if sp_ else None)
        cnt["pe"] += 1
        rec[("mm", gidx)] = cnt["pe"]

        wait("dve", s_pe, rec[("mm", gidx)])
        if kind == "q":
            _, nb, tc = item
            fn = (lambda nb=nb, tc=tc, slot=slot: lambda e: e.tensor_copy(
                qt_sb[:, nb, tc * 512:(tc + 1) * 512], slot[:, :]))()
        elif kind == "k":
            _, nb, tc = item
            fn = (lambda nb=nb, tc=tc, slot=slot: lambda e: e.tensor_copy(
                kt_sb[:, nb, tc * 512:(tc + 1) * 512], slot[:, :]))()
        else:
            _, tb = item
            fn = (lambda tb=tb, slot=slot: lambda e: e.tensor_copy(
                va_sb[:, tb, :, 0:64], slot[:, :]))()
        op("dve", fn, (s_dve, 1))
        cnt["dve"] += 1
        rec[("copy", gidx)] = cnt["dve"]

    # --- logits + exp for one head in one phase ---
    LL = [0]          # global logit tile index
    slot_last = [None] * 3
    # last qt/kt copy gidx needed per (phase, nbh)
    QK_LAST_COPY = {(0, 0): 1, (0, 1): 3, (0, 2): 9, (0, 3): 11,
                    (1, 0): 13, (1, 1): 15, (1, 2): 21, (1, 3): 23}

    def plan_logits(p_, h):
        tiles_p = _phase_tiles(p_)
        for tloc, (jb, i0, w) in enumerate(tiles_p):
            L = LL[0]
            si = L % 3
            slot = ps_lg[si]
            if slot_last[si] is not None:
                wait("pe", s_act, slot_last[si])
            g2, nbh = h % 2, h // 2
            u0 = i0 - 128 * jb
            if L == 0:
                wait("pe", s_d, 32)  # dbias h0-3 + identity loaded
            if p_ == 0 and h == 4 and tloc == 0:
                wait("pe", s_d, 48)  # dbias h4-7 loaded
            if tloc == 0:
                wait("pe", s_dve, rec[("copy", QK_LAST_COPY[(p_, h // 2)])])
            fn = (lambda g2=g2, nbh=nbh, jb=jb, i0=i0, w=w, slot=slot: lambda e: e.matmul(
                slot[:, 0:w],
                kt_sb[64 * g2:64 * g2 + 64, nbh, 128 * jb:128 * jb + 128],
                qt_sb[64 * g2:64 * g2 + 64, nbh, i0:i0 + w],
                start=True, stop=False))()
            op("pe", fn, None)
            fn = (lambda h=h, u0=u0, w=w, slot=slot: lambda e: e.matmul(
                slot[:, 0:w], id_sb[:, :], db_all[:, h, u0:u0 + w],
                start=False, stop=True))()
            op("pe", fn, (s_pe, 1))
            cnt["pe"] += 1
            rec[("lg", L)] = cnt["pe"]

            nsl = 4
            if tloc == 0 and h >= nsl:
                wait("act", s_pe, rec[("avdone", (p_, h - nsl))])
            wait("act", s_pe, rec[("lg", L)])
            fn = (lambda h=h, nsl=nsl, jb=jb, i0=i0, w=w, slot=slot: lambda e: e.activation(
                pt_sb[:, h % nsl, jb, i0:i0 + w], slot[:, 0:w],
                bass.mybir.ActivationFunctionType.Exp, scale=1.0 / 64.0))()
            op("act", fn, (s_act, 1))
            cnt["act"] += 1
            slot_last[si] = cnt["act"]
            LL[0] += 1
        rec[("lgend", (p_, h))] = LL[0]

    # --- AV + normalization drain for one head/phase ---
    CO = [0, 0]  # chunk index, odd-head (tmp) index

    def plan_norm(Cn):
        if ("norm", Cn) in rec:
            return
        pn, hn = rec[("ph", Cn)]
        csn = Cn % 2
        if rec[("peb", Cn)]:
            pend_dve.append(("wait", s_bc, rec[("bccp", Cn)]))
        else:
            pend_dve.append(("wait", s_rbc[csn], rec[("bcv", Cn)]))
        in1 = rbc[csn][:, :]
        slotn = ps_at[Cn % 3]
        if hn % 2 == 0:
            fn = (lambda pn=pn, hn=hn, slotn=slotn, in1=in1: lambda e: e.tensor_mul(
                at_sb[0:64, hn // 2, pn * 512:(pn + 1) * 512], slotn[0:64, :], in1))()
            pend_dve.append(("op", fn, (s_dr, 1)))
        else:
            On = rec[("oidx", Cn)]
            ts = On % 2
            if On >= 2:
                pend_dve.append(("wait", s_tm[ts], 16 * (On // 2)))  # tmp slot free
            fn = (lambda ts=ts, slotn=slotn, in1=in1: lambda e: e.tensor_mul(
                tmp[ts][:, :], slotn[0:64, :], in1))()
            pend_dve.append(("op", fn, (s_dr, 1)))
        cnt["dr"] += 1
        rec[("norm", Cn)] = cnt["dr"]
        if hn % 2 == 1:
            On = rec[("oidx", Cn)]
            ts = On % 2
            wait("gp", s_dr, rec[("norm", Cn)])
            fn = (lambda pn=pn, hn=hn, ts=ts: lambda e: e.dma_start(
                out=at_sb[64:128, hn // 2, pn * 512:(pn + 1) * 512], in_=tmp[ts][:, :]))()
            op("gp", fn, (s_tm[ts], 16))
            cnt["tm"][ts] += 1
            rec[("tmdma", Cn)] = (ts, 16 * cnt["tm"][ts])
            tmch[pn].append(Cn)

    srow_free = {0: None, 1: None}
    pend_dve = []
    pend_pe = []
    pend_act = []

    def flush_dve():
        ops["dve"].extend(pend_dve)
        pend_dve.clear()

    def flush_pe():
        ops["pe"].extend(pend_pe)
        pend_pe.clear()

    def flush_act():
        ops["act"].extend(pend_act)
        pend_act.clear()

    def plan_av(p_, h, defer_norm=False):
        flush_pe()
        flush_act()
        C = CO[0]
        peb = C >= 6
        rec[("ph", C)] = (p_, h)
        rec[("peb", C)] = peb
        slot = ps_at[C % 3]
        if C == 0:
            wait("pe", s_dve, rec[("copy", 7)])   # va tb0-3 ready
        if C == 8:
            wait("pe", s_dve, rec[("copy", 19)])  # va tb4-7 ready
        if C >= 3:
            wait("pe", s_dr, rec[("norm", C - 3)])
        njb = 4 if p_ == 0 else 8
        nsl = 4
        lgstart = rec[("lgend", (p_, h))] - njb
        for k, jb in enumerate(range(njb)):
            st, sp_ = k == 0, k == njb - 1
            wait("pe", s_act, lgstart + jb + 1)
            fn = (lambda h=h, nsl=nsl, jb=jb, p_=p_, slot=slot, st=st, sp_=sp_: lambda e: e.matmul(
                slot[:, :], va_sb[:, jb, h, 0:65], pt_sb[:, h % nsl, jb, p_ * 512:(p_ + 1) * 512],
                start=st, stop=sp_))()
            op("pe", fn, (s_pe, 1) if sp_ else None)
        cnt["pe"] += 1
        rec[("av", C)] = cnt["pe"]
        rec[("avdone", (p_, h))] = cnt["pe"]

        cs = C % 2
        if peb:
            # DVE: reciprocal of the whole sum row straight out of PSUM
            wait("dve", s_pe, rec[("av", C)])
            if srow_free[cs] is not None:
                wait("dve", srow_free[cs][0], srow_free[cs][1])
            op("dve", (lambda cs=cs, slot=slot: lambda e: e.reciprocal(
                srow[64:65, cs, 0:256], slot[64:65, 0:256]))(), (s_rp, 1))
            cnt["rp"] += 1
            rec[("rcpA", C)] = cnt["rp"]
            op("dve", (lambda cs=cs, slot=slot: lambda e: e.reciprocal(
                srow[64:65, cs, 256:512], slot[64:65, 256:512]))(), (s_rp, 1))
            cnt["rp"] += 1
            rec[("rcp", C)] = cnt["rp"]
            flush_dve()  # prior norms queue behind the critical-path recips

            # PE: broadcast 1/s across 64 partitions via ones-stationary
            # matmul; deferred in the PE queue so the row-reciprocal latency
            # hides under the next logits/AV work
            pend_pe.append(("wait", s_rp, rec[("rcpA", C)]))
            if C == 6:
                pend_pe.append(("wait", s_dve, rec[("copy", 22)]))
            elif C == 7:
                pend_pe.append(("wait", s_dve, rec[("copy", 23)]))
            elif C == 8:
                pend_pe.append(("wait", s_dve, rec[("stage", 6)]))
            elif C == 9:
                pend_pe.append(("wait", s_dve, rec[("stage", 7)]))
            else:
                pend_pe.append(("wait", s_bc, rec[("bccp", C - 2)]))
            fn = (lambda C=C, cs=cs: lambda e: e.matmul(
                ps_mm[C % 2][0:64, 0:256], on_sb[64:65, 0:64], srow[64:65, cs, 0:256],
                start=True, stop=True))()
            pend_pe.append(("op", fn, (s_bm, 1)))
            cnt["bm"] += 1
            pend_pe.append(("wait", s_rp, rec[("rcp", C)]))
            fn = (lambda C=C, cs=cs: lambda e: e.matmul(
                ps_mm[C % 2][0:64, 256:512], on_sb[64:65, 0:64], srow[64:65, cs, 256:512],
                start=True, stop=True))()
            pend_pe.append(("op", fn, (s_bm, 1)))
            cnt["bm"] += 1
            rec[("bcmm", C)] = cnt["bm"]
            srow_free[cs] = (s_bm, cnt["bm"])

            # Copy the broadcast out of PSUM (DVE can't read 2 PSUM inputs).
            # Mid-phase chunks: on ACT, deferred past the exps PE still needs.
            # Phase-end chunks: ph0 on DVE (act busy with next-phase exps),
            # ph1 on ACT directly (act idle there).
            fn = (lambda C=C, cs=cs: lambda e: e.copy(
                rbc[cs][:, :], ps_mm[C % 2][0:64, 0:512]))()
            if C in (6, 7):
                fn = (lambda C=C, cs=cs: lambda e: e.tensor_copy(
                    rbc[cs][:, :], ps_mm[C % 2][0:64, 0:512]))()
                wait("dve", s_bm, rec[("bcmm", C)])
                if C >= 2 and ("norm", C - 2) in rec:
                    wait("dve", s_dr, rec[("norm", C - 2)])
                op("dve", fn, (s_bc, 1))
            elif C >= 14:
                wait("act", s_bm, rec[("bcmm", C)])
                if ("norm", C - 2) in rec:
                    wait("act", s_dr, rec[("norm", C - 2)])
                op("act", fn, (s_bc, 1))
            else:
                pend_act.append(("wait", s_bm, rec[("bcmm", C)]))
                if ("norm", C - 2) in rec:
                    pend_act.append(("wait", s_dr, rec[("norm", C - 2)]))
                pend_act.append(("op", fn, (s_bc, 1)))
            cnt["bc"] += 1
            rec[("bccp", C)] = cnt["bc"]
        else:
            # DVE: copy the sum row out of PSUM
            wait("dve", s_pe, rec[("av", C)])
            if srow_free[cs] is not None:
                wait("dve", srow_free[cs][0], srow_free[cs][1])
            op("dve", (lambda cs=cs, slot=slot: lambda e: e.tensor_copy(
                srow[64:65, cs, :], slot[64:65, :]))(), (s_dve, 1))
            cnt["dve"] += 1
            rec[("rowcp", C)] = cnt["dve"]
            flush_dve()

            # GP: columnarize row -> sP[128, 4] (sbuf->sbuf)
            wait("gp", s_dve, rec[("rowcp", C)])
            wait("gp", s_r2[cs], 16 * cnt["r2"][cs])  # sP slot free
            op("gp", (lambda cs=cs: lambda e: e.dma_start(
                out=sP[:, cs, :], in_=srow[64:65, cs, :]))(), (s_rd[cs], 16))
            cnt["rd"][cs] += 1
            srow_free[cs] = (s_rd[cs], 16 * cnt["rd"][cs])

            # DVE: reciprocal on [128, 4] (free size 4: fast), deferred in the
            # queue so its col-DMA completion wait never blocks QKV copies
            pend_dve.append(("wait", s_rd[cs], 16 * cnt["rd"][cs]))
            pend_dve.append(("op", (lambda cs=cs: lambda e: e.reciprocal(
                sP[:, cs, :], sP[:, cs, :]))(), (s_rp, 1)))
            cnt["rp"] += 1
            rec[("rcp", C)] = cnt["rp"]

            # GP: sP -> r2_dram -> broadcast to rbc[64, 512]
            wait("gp", s_rp, rec[("rcp", C)])
            wait("gp", s_rbc[cs], 16 * cnt["rbc"][cs])  # r2_dram slot free
            op("gp", (lambda cs=cs: lambda e: e.dma_start(
                out=r2_dram[cs, :, :], in_=sP[:, cs, :]))(), (s_r2[cs], 16))
            cnt["r2"][cs] += 1
            wait("gp", s_r2[cs], 16 * cnt["r2"][cs])
            if C >= 2:
                wait("gp", s_dr, rec[("norm", C - 2)])  # rbc slot free
            op("gp", (lambda cs=cs: lambda e: e.dma_start(
                out=rbc[cs][:, :],
                in_=r2_dram[cs, None, :, :].broadcast_to([64, 128, 4])))(), (s_rbc[cs], 16))
            cnt["rbc"][cs] += 1
            rec[("bcv", C)] = 16 * cnt["rbc"][cs]

        if h % 2 == 1:
            rec[("oidx", C)] = CO[1]
            CO[1] += 1
        if C >= 1 and not defer_norm:
            plan_norm(C - 1)
        CO[0] += 1

    # --- output projection quarter q + RS ---
    OG = [0]

    def plan_oproj_quarter(q):
        flush_dve()
        flush_pe()
        flush_act()
        p_ = 0 if q < 2 else 1
        groups = [(tb, mc) for tb in Q_TBS[q] for mc in range(2)]
        for j, (tb, mc) in enumerate(groups):
            og = OG[0]
            slot = ps_mm[og % 2]
            if og == 0:
                wait("pe", s_wo, 16)
                wait("pe", s_dve, rec[("copy", 23)])  # ps_mm free of QKV
            if j == 0 and q in (0, 2):
                # all norms of phase p_ done (incl. odd-head tmp DMAs)
                wait("pe", s_dr, rec[("norm", 8 * p_ + 7)])
                for Cn in tmch[p_]:
                    ts, tv = rec[("tmdma", Cn)]
                    wait("pe", s_tm[ts], tv)
            if og >= 2:
                wait("pe", s_dve, rec[("stage", og - 2)])
            for nb in range(4):
                st, sp_ = nb == 0, nb == 3
                fn = (lambda nb=nb, tb=tb, mc=mc, slot=slot, st=st, sp_=sp_: lambda e: e.matmul(
                    slot[:, :], at_sb[:, nb, tb * 128:(tb + 1) * 128], wo_sb[:, nb, mc * 512:(mc + 1) * 512],
                    start=st, stop=sp_))()
                op("pe", fn, (s_pe, 1) if sp_ else None)
            cnt["pe"] += 1
            rec[("op", og)] = cnt["pe"]

            wait("dve", s_pe, rec[("op", og)])
            if og >= 8:
                wait("dve", s_out, 16 * (og - 7))  # stg slot free
            fn = (lambda og=og, slot=slot: lambda e: e.tensor_copy(
                stg[:, og % 8, :], slot[:, :]))()
            op("dve", fn, (s_dve, 1))
            cnt["dve"] += 1
            rec[("stage", og)] = cnt["dve"]

            wait("sp", s_dve, rec[("stage", og)])
            pdst = partials[q]
            fn = (lambda j=j, og=og, pdst=pdst: lambda e: e.dma_start(
                out=pdst[(j // 2) * 128:(j // 2 + 1) * 128, (j % 2) * 512:(j % 2 + 1) * 512],
                in_=stg[:, og % 8, :]))()
            op("sp", fn, (s_out, 16))
            cnt["out"] += 1
            OG[0] += 1

        wait("gp", s_out, 16 * cnt["out"])
        op("gp", (lambda q=q: lambda e: e.collective_compute(
            "ReduceScatter", bass.mybir.AluOpType.add,
            replica_groups=[[0, 1], [2, 3], [4, 5], [6, 7]],
            ins=[partials[q].ap().opt()], outs=[reds[q].ap().opt()]))(), (s_cc, 1))

    # ---- master schedule ----
    qkv_groups = (
        [("q", 0, 0), ("k", 0, 0)],                             # A: heads 0,1 (t first half)
        [("q", 1, 0), ("k", 1, 0)],                             # B: heads 2,3
        [("v", 0), ("v", 1), ("v", 2), ("v", 3)],               # C
        [("q", 2, 0), ("k", 2, 0)],                             # D: heads 4,5
        [("q", 3, 0), ("k", 3, 0)],                             # E: heads 6,7
        [("q", 0, 1), ("k", 0, 1), ("q", 1, 1), ("k", 1, 1)],   # F: tc1 nb0/1
        [("v", 4), ("v", 5), ("v", 6), ("v", 7)],               # G
        [("q", 2, 1), ("k", 2, 1), ("q", 3, 1), ("k", 3, 1)],   # H: tc1 nb2/3
    )
    gi = [0]

    def emit_qkv(block):
        for item in qkv_groups[block]:
            g = gi[0]
            if item[0] == "q":
                if g == 0:
                    wait("pe", s_xT, 32)
                    wait("pe", s_wq, 16)
                if item[1] == 1 and item[2] == 0:
                    wait("pe", s_wq, 32)
                if item[2] == 1 and item[1] == 0:
                    wait("pe", s_xT, 48)
            elif item[0] == "k":
                if item[1] == 0 and item[2] == 0:
                    wait("pe", s_wk, 16)
                if item[1] == 1 and item[2] == 0:
                    wait("pe", s_wk, 32)
            else:
                if item[1] == 0:
                    wait("pe", s_wv, 16)
            plan_qkv(item, g)
            gi[0] += 1

    emit_qkv(0)                      # g0-1
    plan_logits(0, 0)
    plan_logits(0, 1)
    emit_qkv(1)                      # g2-3
    plan_logits(0, 2)
    plan_logits(0, 3)
    emit_qkv(2)                      # g4-7: v tb0-3
    plan_av(0, 0)
    emit_qkv(3)                      # g8-9
    plan_av(0, 1)
    plan_logits(0, 4)
    plan_av(0, 3)
    plan_logits(0, 5)
    emit_qkv(4)                      # g10-11
    plan_av(0, 2)
    plan_logits(0, 6)
    emit_qkv(5)                      # g12-15: tc1 nb0/1
    plan_av(0, 5)
    plan_logits(0, 7)
    plan_av(0, 7)                    # C5: last odd head retired early
    emit_qkv(6)                      # g16-19: v tb4-7
    emit_qkv(7)                      # g20-23: tc1 nb2/3
    plan_av(0, 4)                    # C6: PE-broadcast drain (ps_mm free)
    plan_av(0, 6, defer_norm=True)   # C7: even heads at the gate, no tm DMA
    plan_norm(6)                     # drain phase 0
    plan_norm(7)

    # phase 1: odd heads (which need the extra tmp->at DMA) run early so the
    # tail drains on even heads only
    plan_logits(1, 0)
    plan_logits(1, 1)
    plan_logits(1, 2)
    plan_logits(1, 3)
    plan_oproj_quarter(0)
    plan_oproj_quarter(1)
    plan_av(1, 1)                    # C8
    plan_logits(1, 5)
    plan_av(1, 3)                    # C9
    plan_logits(1, 7)
    plan_av(1, 2)                    # C10
    plan_logits(1, 6)
    plan_av(1, 5)                    # C11
    plan_av(1, 7)                    # C12
    plan_av(1, 0)                    # C13
    plan_logits(1, 4)
    plan_av(1, 6)                    # C14
    plan_av(1, 4, defer_norm=True)   # C15
    plan_norm(14)                    # drain phase 1
    plan_norm(15)
    plan_oproj_quarter(2)
    plan_oproj_quarter(3)

    for q in range(4):
        wait("sp", s_cc, q + 2)  # +1 for the warm-up collective
        op("sp", (lambda q=q: lambda e: e.dma_start(
            out=outs_p[q][:, :], in_=reds[q][:, :]))(), (s_fin, 16))
    wait("gp", s_fin, 64)

    _CACHE["ops_debug"] = {k: list(v) for k, v in ops.items()}

    # ---- emit ----
    def emit(eng, lst):
        for item in lst:
            if item[0] == "wait":
                eng.wait_ge(item[1], item[2])
            else:
                inst = item[1](eng)
                if item[2] is not None:
                    inst.then_inc(item[2][0], item[2][1])

    with nc.allow_low_precision("bf16 softmax normalization broadcast"), nc.Block() as block:
        @block.sync
        def _(e):
            emit(e, ops["sp"])

        @block.tensor
        def _(e):
            emit(e, ops["pe"])

        @block.vector
        def _(e):
            emit(e, ops["dve"])

        @block.scalar
        def _(e):
            emit(e, ops["act"])

        @block.gpsimd
        def _(e):
            emit(e, ops["gp"])

    ctx.close()
    return nc


def _get_nc():
    if "nc" not in _CACHE:
        _CACHE["nc"] = _build()
    return _CACHE["nc"]


def _prep_inputs(x, Wq, Wk, Wv, Wo, bo, rel_pos_bias):
    bf = ml_dtypes.bfloat16
    in_maps = []
    p_idx = np.arange(128)[:, None]
    u_idx = np.arange(T)[None, :]
    for core in range(NCORES):
        b, g = core // 2, core % 2
        xb = np.asarray(x[b], dtype=np.float32)
        xT_h = np.ascontiguousarray(
            xb.T.reshape(8, 128, T).transpose(1, 0, 2)).astype(bf)
        wq_h = np.ascontiguousarray(
            Wq[:, g * NL:(g + 1) * NL].reshape(8, 128, NL).transpose(1, 0, 2)).astype(bf)
        wk_h = np.ascontiguousarray(
            Wk[:, g * NL:(g + 1) * NL].reshape(8, 128, NL).transpose(1, 0, 2)).astype(bf)
        wv_h = np.ascontiguousarray(
            Wv[:, g * NL:(g + 1) * NL].reshape(8, 128, NL).transpose(1, 0, 2)).astype(bf)
        wo_h = np.ascontiguousarray(
            Wo[g * NL:(g + 1) * NL, :].reshape(4, 128, D).transpose(1, 0, 2)).astype(bf)
        db = np.empty((128, HL, T), dtype=bf)
        dif = np.clip(u_idx - p_idx, 0, T - 1)
        msk = u_idx >= p_idx
        for h in range(HL):
            rev = np.asarray(rel_pos_bias[g * HL + h], dtype=np.float32)[::-1]
            db[:, h, :] = np.where(msk, rev[dif], NEG).astype(bf)
        in_maps.append({
            "xT": xT_h,
            "wq0": np.ascontiguousarray(wq_h[:, :, 0:128]),
            "wq1": np.ascontiguousarray(wq_h[:, :, 128:512]),
            "wk0": np.ascontiguousarray(wk_h[:, :, 0:128]),
            "wk1": np.ascontiguousarray(wk_h[:, :, 128:512]),
            "wv": wv_h, "wo": wo_h,
            "dbias": db, "ident": np.eye(128, dtype=bf),
        })
    return in_maps


def run_on_device(x, Wq, Wk, Wv, Wo, bo, rel_pos_bias, trace=False):
    from concourse.bass_utils import run_bass_kernel_spmd

    nc = _get_nc()
    in_maps = _prep_inputs(x, Wq, Wk, Wv, Wo, bo, rel_pos_bias)
    res = run_bass_kernel_spmd(nc, in_maps, core_ids=list(range(NCORES)), trace=trace)
    bo_f = np.asarray(bo, np.float32)
    outs = []
    for b in range(B):
        ev = res.results[2 * b]
        od = res.results[2 * b + 1]
        rows = []
        for q in range(4):
            rows.append(ev[f"out{q}"])
            rows.append(od[f"out{q}"])
        outs.append(np.concatenate(rows, axis=0))
    out = np.stack(outs).astype(np.float32) + bo_f[None, None, :]
    return out, res


def kernel(x, Wq, Wk, Wv, Wo, bo, rel_pos_bias):
    out, _ = run_on_device(x, Wq, Wk, Wv, Wo, bo, rel_pos_bias, trace=False)
    return out


# revision 90
# speedup vs baseline: 1.0271x; 1.0271x over previous
"""Self-contained Trainium2 Bass kernel for causal attention with relative
position bias (B=4, T=1024, D=1024, H=16, dh=64), SPMD across 8 NeuronCores.

Sharding: core = (batch b = core//2, head-half g = core%2). Each core computes
QKV projections for its 8 heads, causal attention, and a partial output
projection; partials are summed pairwise with on-device ReduceScatters (one
per T/4 output quarter, written directly into output parameters).

Schedule: QKV projection groups are interleaved with per-head logits/exp/AV so
the Scalar engine's exp work overlaps the projection matmuls. Softmax
normalization: for early chunks (C0-C5, while ps_mm is busy with QKV) the
reciprocal row is columnarized through a DRAM bounce and broadcast back by
DMA; for later chunks a DVE row-reciprocal (in two halves) is broadcast
across 64 partitions by a tiny ones-stationary PE matmul into the free ps_mm
bank, then copied to SBUF and multiplied in. Drain ops are deferred in each
engine queue (with dedicated semaphores) so their cross-engine waits never
block QKV copies or the PE stream. Outputs leave via three ReduceScatters
(t rows [0:256), [256:512), [512:1024)) with a warm-up dummy collective to
absorb the CC cold start; bo is added on the host after gathering.

Layouts (per core):
  xT    [128, 8, 1024]  bf16   x[b].T as [d%128, d//128, t]
  wq/wk [128, 8, 512]   bf16   W[:, g*512:+512] as [d%128, d//128, n]
  wv    [128, 8, 512]   bf16   same
  wo    [128, 4, 1024]  bf16   Wo[g*512:+512, :] as [n%128, n//128, m]
  dbias [128, 8, 1024]  bf16   per local head: bias[j%128, i-128jb] with
                               causal mask folded in as -1e9
  QT/KT [128, 4, 1024]  bf16   [n%128, n//128, t]  (channel-major)
  V_aug [128, 8, 8, 65] bf16   [t%128, t//128, h, c] with ones column c=64
  pt    [128, 2, 8, 1024] bf16 exp((logits+bias)/64), [j%128, jb, i]
  attT  [128, 4, 1024]  bf16   normalized attention output, channel-major
"""
import sys

sys.path.insert(0, "/opt/trn_rl_repo")

import numpy as np
import ml_dtypes

B, T, D = 4, 1024, 1024
H, DH = 16, 64
HL, NL = 8, 512  # local heads / channels per core
NCORES = 8
NEG = -1.0e9

_CACHE = {}


def _phase_tiles(p):
    """Logit tiles (jb, i0, w) for query phase p (cols 512p..512p+512)."""
    tiles = []
    if p == 0:
        for jb in range(4):
            tiles.append((jb, 128 * jb, 512 - 128 * jb))
    else:
        for jb in range(8):
            i0 = max(512, 128 * jb)
            tiles.append((jb, i0, 1024 - i0))
    return tiles


def _build():
    from concourse import bass
    from contextlib import ExitStack

    mybir = bass.mybir
    f32, bf16 = mybir.dt.float32, mybir.dt.bfloat16

    nc = bass.Bass(target_bir_lowering=False, debug=False)
    xT = nc.declare_dram_parameter("xT", [128, 8, T], bf16, isOutput=False)
    wq0 = nc.declare_dram_parameter("wq0", [128, 8, 128], bf16, isOutput=False)
    wq1 = nc.declare_dram_parameter("wq1", [128, 8, 384], bf16, isOutput=False)
    wk0 = nc.declare_dram_parameter("wk0", [128, 8, 128], bf16, isOutput=False)
    wk1 = nc.declare_dram_parameter("wk1", [128, 8, 384], bf16, isOutput=False)
    wv = nc.declare_dram_parameter("wv", [128, 8, NL], bf16, isOutput=False)
    wo = nc.declare_dram_parameter("wo", [128, 4, D], bf16, isOutput=False)
    dbias = nc.declare_dram_parameter("dbias", [128, HL, T], bf16, isOutput=False)
    ident = nc.declare_dram_parameter("ident", [128, 128], bf16, isOutput=False)
    Q_TBS = [[0, 1], [2, 3], [4, 5, 6, 7]]  # t-blocks per output chunk
    outs_p = [nc.declare_dram_parameter(f"out{q}", [64 * len(t), D], bf16, isOutput=True)
              for q, t in enumerate(Q_TBS)]

    partials = [nc.dram_tensor(f"partial{q}", [128 * len(t), D], bf16)
                for q, t in enumerate(Q_TBS)]
    reds = [nc.dram_tensor(f"red{q}", [64 * len(t), D], bf16)
            for q, t in enumerate(Q_TBS)]
    warm_in = nc.dram_tensor("warm_in", [2, 64], bf16)
    warm_out = nc.dram_tensor("warm_out", [1, 64], bf16)
    r2_dram = nc.dram_tensor("r2_dram", [2, 128, 4], f32)

    ctx = ExitStack()
    sem = lambda n: ctx.enter_context(nc.semaphore(n))
    sb = lambda n, shape, dt: ctx.enter_context(nc.sbuf_tensor(n, shape, dt))
    ps = lambda n, shape: ctx.enter_context(nc.psum_tensor(n, shape, f32))

    s_xT = sem("s_xT")
    s_wq = sem("s_wq")
    s_wk = sem("s_wk")
    s_wv = sem("s_wv")
    s_wo = sem("s_wo")
    s_d = sem("s_d")
    s_pe = sem("s_pe")
    s_dve = sem("s_dve")
    s_dr = sem("s_dr")
    s_act = sem("s_act")
    s_rd = [sem("s_rd0"), sem("s_rd1")]
    s_r2 = [sem("s_r20"), sem("s_r21")]
    s_rbc = [sem("s_rbc0"), sem("s_rbc1")]
    s_tm = [sem("s_tm0"), sem("s_tm1")]
    s_out = sem("s_out")
    s_cc = sem("s_cc")
    s_fin = sem("s_fin")
    s_bc = sem("s_bc")
    s_rp = sem("s_rp")
    s_bm = sem("s_bm")

    xT_sb = sb("xT_sb", [128, 8, T], bf16)
    wq_sb = sb("wq_sb", [128, 8, NL], bf16)
    wk_sb = sb("wk_sb", [128, 8, NL], bf16)
    wv_sb = sb("wv_sb", [128, 8, NL], bf16)
    wo_sb = sb("wo_sb", [128, 4, D], bf16)
    qt_sb = sb("qt_sb", [128, 4, T], bf16)
    kt_sb = sb("kt_sb", [128, 4, T], bf16)
    va_sb = sb("va_sb", [128, 8, HL, 65], bf16)
    pt_sb = sb("pt_sb", [128, 4, 8, T], bf16)  # ph0: slot h%4, ph1: slot h%2
    at_sb = sb("at_sb", [128, 4, T], bf16)
    db_all = sb("db_all", [128, HL, T], bf16)
    id_sb = sb("id_sb", [128, 128], bf16)
    srow = sb("srow", [65, 2, 512], f32)  # recip rows live on partition 64
    sP = sb("sP", [128, 2, 4], f32)
    on_sb = sb("on_sb", [65, 64], f32)    # ones row for the PE broadcast
    rbc = [sb("rbc0", [64, 512], f32), sb("rbc1", [64, 512], f32)]
    tmp = [sb("tmp0", [64, 512], bf16), sb("tmp1", [64, 512], bf16)]
    stg = sb("stg", [128, 8, 512], bf16)

    ps_mm = [ps("ps_mm0", [128, 512]), ps("ps_mm1", [128, 512])]
    ps_lg = [ps("ps_lg0", [128, 512]), ps("ps_lg1", [128, 512]), ps("ps_lg2", [128, 512])]
    ps_at = [ps("ps_at0", [65, 512]), ps("ps_at1", [65, 512]), ps("ps_at2", [65, 512])]

    # ---- plan ----
    ops = {k: [] for k in ("sp", "pe", "dve", "act", "gp")}

    def wait(eng, s, v):
        ops[eng].append(("wait", s, v))

    def op(eng, fn, inc=None):
        ops[eng].append(("op", fn, inc))

    cnt = {"pe": 0, "dve": 0, "act": 0, "dr": 0, "rp": 0, "bm": 0, "out": 0, "bc": 0,
           "rd": [0, 0], "r2": [0, 0], "rbc": [0, 0], "tm": [0, 0]}
    tmch = {0: [], 1: []}  # chunks with tm DMAs per phase
    rec = {}

    # --- input DMAs: critical loads split across both HWDGE queues ---
    op("sp", lambda e: e.dma_start(out=xT_sb[:, :, 0:256], in_=xT[:, :, 0:256]), (s_xT, 16))
    op("sp", lambda e: e.dma_start(out=xT_sb[:, :, 256:512], in_=xT[:, :, 256:512]), (s_xT, 16))
    op("sp", lambda e: e.dma_start(out=db_all[:, 0:4, :], in_=dbias[:, 0:4, :]), (s_d, 16))
    op("sp", lambda e: e.dma_start(out=id_sb[:], in_=ident[:, :]), (s_d, 16))
    op("sp", lambda e: e.dma_start(out=wv_sb[:], in_=wv[:]), (s_wv, 16))
    op("sp", lambda e: e.dma_start(out=db_all[:, 4:8, :], in_=dbias[:, 4:8, :]), (s_d, 16))
    op("sp", lambda e: e.dma_start(out=xT_sb[:, :, 512:1024], in_=xT[:, :, 512:1024]), (s_xT, 16))
    op("sp", lambda e: e.dma_start(out=wo_sb[:], in_=wo[:]), (s_wo, 16))

    op("act", lambda e: e.dma_start(out=wq_sb[:, :, 0:128], in_=wq0[:]), (s_wq, 16))
    op("act", lambda e: e.dma_start(out=wk_sb[:, :, 0:128], in_=wk0[:]), (s_wk, 16))
    op("act", lambda e: e.dma_start(out=wq_sb[:, :, 128:512], in_=wq1[:]), (s_wq, 16))
    op("act", lambda e: e.dma_start(out=wk_sb[:, :, 128:512], in_=wk1[:]), (s_wk, 16))

    # cold-start the collective engine with a tiny dummy ReduceScatter
    op("gp", lambda e: e.collective_compute(
        "ReduceScatter", bass.mybir.AluOpType.add,
        replica_groups=[[0, 1], [2, 3], [4, 5], [6, 7]],
        ins=[warm_in.ap().opt()], outs=[warm_out.ap().opt()]), (s_cc, 1))

    # --- DVE memsets: pt invalid regions + V ones column ---
    for sl in range(4):          # ph0 (cols 0:512): jb 1-3 need leading zeros
        for jb in range(1, 4):
            op("dve", (lambda sl=sl, j=jb: lambda e: e.memset(pt_sb[:, sl, j, 0:128 * j], 0.0))(), (s_dve, 1))
            cnt["dve"] += 1
    for sl in range(4):          # ph1 (cols 512:1024): jb 5-7 need leading zeros
        for jb in range(5, 8):
            op("dve", (lambda sl=sl, j=jb: lambda e: e.memset(pt_sb[:, sl, j, 512:128 * j], 0.0))(), (s_dve, 1))
            cnt["dve"] += 1
    op("dve", lambda e: e.memset(va_sb[:, :, :, 64:65], 1.0), (s_dve, 1))
    cnt["dve"] += 1
    op("dve", lambda e: e.memset(on_sb[64:65, :], 1.0), (s_dve, 1))
    cnt["dve"] += 1

    # HAM warm-up: dummy matmuls on untouched SBUF during the input-DMA wait.
    for _w in range(14):
        op("pe", (lambda: lambda e: e.matmul(
            ps_lg[0][:, :], stg[:, 0, 0:128], stg[:, 1, :],
            start=True, stop=True))(), None)

    # --- QKV projection groups ---
    def plan_qkv(item, gidx):
        kind = item[0]
        slot = ps_mm[gidx % 2]
        if gidx >= 2:
            wait("pe", s_dve, rec[("copy", gidx - 2)])
        for db in range(8):
            st, sp_ = db == 0, db == 7
            if kind in ("q", "k"):
                _, nb, tc = item
                w = wq_sb if kind == "q" else wk_sb
                fn = (lambda w=w, nb=nb, tc=tc, db=db, slot=slot, st=st, sp_=sp_: lambda e: e.matmul(
                    slot[:, :], w[:, db, nb * 128:(nb + 1) * 128], xT_sb[:, db, tc * 512:(tc + 1) * 512],
                    start=st, stop=sp_))()
            else:
                _, tb = item
                fn = (lambda tb=tb, db=db, slot=slot, st=st, sp_=sp_: lambda e: e.matmul(
                    slot[:, :], xT_sb[:, db, tb * 128:(tb + 1) * 128], wv_sb[:, db, 0:NL],
                    start=st, stop=sp_))()
            op("pe", fn, (s_pe, 1) if sp_ else None)
        cnt["pe"] += 1
        rec[("mm", gidx)] = cnt["pe"]

        wait("dve", s_pe, rec[("mm", gidx)])
        if kind == "q":
            _, nb, tc = item
            fn = (lambda nb=nb, tc=tc, slot=slot: lambda e: e.tensor_copy(
                qt_sb[:, nb, tc * 512:(tc + 1) * 512], slot[:, :]))()
        elif kind == "k":
            _, nb, tc = item
            fn = (lambda nb=nb, tc=tc, slot=slot: lambda e: e.tensor_copy(
                kt_sb[:, nb, tc * 512:(tc + 1) * 512], slot[:, :]))()
        else:
            _, tb = item
            fn = (lambda tb=tb, slot=slot: lambda e: e.tensor_copy(
                va_sb[:, tb, :, 0:64], slot[:, :]))()
        op("dve", fn, (s_dve, 1))
        cnt["dve"] += 1
        rec[("copy", gidx)] = cnt["dve"]

    # --- logits + exp for one head in one phase ---
    LL = [0]          # global logit tile index
    slot_last = [None] * 3
    # last qt/kt copy gidx needed per (phase, nbh)
    QK_LAST_COPY = {(0, 0): 1, (0, 1): 3, (0, 2): 9, (0, 3): 11,
                    (1, 0): 13, (1, 1): 15, (1, 2): 21, (1, 3): 23}

    def plan_logits(p_, h):
        tiles_p = _phase_tiles(p_)
        for tloc, (jb, i0, w) in enumerate(tiles_p):
            L = LL[0]
            si = L % 3
            slot = ps_lg[si]
            if slot_last[si] is not None:
                wait("pe", s_act, slot_last[si])
            g2, nbh = h % 2, h // 2
            u0 = i0 - 128 * jb
            if L == 0:
                wait("pe", s_d, 32)  # dbias h0-3 + identity loaded
            if p_ == 0 and h == 4 and tloc == 0:
                wait("pe", s_d, 48)  # dbias h4-7 loaded
            if tloc == 0:
                wait("pe", s_dve, rec[("copy", QK_LAST_COPY[(p_, h // 2)])])
            fn = (lambda g2=g2, nbh=nbh, jb=jb, i0=i0, w=w, slot=slot: lambda e: e.matmul(
                slot[:, 0:w],
                kt_sb[64 * g2:64 * g2 + 64, nbh, 128 * jb:128 * jb + 128],
                qt_sb[64 * g2:64 * g2 + 64, nbh, i0:i0 + w],
                start=True, stop=False))()
            op("pe", fn, None)
            fn = (lambda h=h, u0=u0, w=w, slot=slot: lambda e: e.matmul(
                slot[:, 0:w], id_sb[:, :], db_all[:, h, u0:u0 + w],
                start=False, stop=True))()
            op("pe", fn, (s_pe, 1))
            cnt["pe"] += 1
            rec[("lg", L)] = cnt["pe"]

            nsl = 4
            if tloc == 0 and h >= nsl:
                wait("act", s_pe, rec[("avdone", (p_, h - nsl))])
            wait("act", s_pe, rec[("lg", L)])
            fn = (lambda h=h, nsl=nsl, jb=jb, i0=i0, w=w, slot=slot: lambda e: e.activation(
                pt_sb[:, h % nsl, jb, i0:i0 + w], slot[:, 0:w],
                bass.mybir.ActivationFunctionType.Exp, scale=1.0 / 64.0))()
            op("act", fn, (s_act, 1))
            cnt["act"] += 1
            slot_last[si] = cnt["act"]
            LL[0] += 1
        rec[("lgend", (p_, h))] = LL[0]

    # --- AV + normalization drain for one head/phase ---
    CO = [0, 0]  # chunk index, odd-head (tmp) index

    def plan_norm(Cn):
        if ("norm", Cn) in rec:
            return
        pn, hn = rec[("ph", Cn)]
        csn = Cn % 2
        if rec[("peb", Cn)]:
            pend_dve.append(("wait", s_bc, rec[("bccp", Cn)]))
        else:
            pend_dve.append(("wait", s_rbc[csn], rec[("bcv", Cn)]))
        in1 = rbc[csn][:, :]
        slotn = ps_at[Cn % 3]
        if hn % 2 == 0:
            fn = (lambda pn=pn, hn=hn, slotn=slotn, in1=in1: lambda e: e.tensor_mul(
                at_sb[0:64, hn // 2, pn * 512:(pn + 1) * 512], slotn[0:64, :], in1))()
            pend_dve.append(("op", fn, (s_dr, 1)))
        else:
            On = rec[("oidx", Cn)]
            ts = On % 2
            if On >= 2:
                pend_dve.append(("wait", s_tm[ts], 16 * (On // 2)))  # tmp slot free
            fn = (lambda ts=ts, slotn=slotn, in1=in1: lambda e: e.tensor_mul(
                tmp[ts][:, :], slotn[0:64, :], in1))()
            pend_dve.append(("op", fn, (s_dr, 1)))
        cnt["dr"] += 1
        rec[("norm", Cn)] = cnt["dr"]
        if hn % 2 == 1:
            On = rec[("oidx", Cn)]
            ts = On % 2
            wait("gp", s_dr, rec[("norm", Cn)])
            fn = (lambda pn=pn, hn=hn, ts=ts: lambda e: e.dma_start(
                out=at_sb[64:128, hn // 2, pn * 512:(pn + 1) * 512], in_=tmp[ts][:, :]))()
            op("gp", fn, (s_tm[ts], 16))
            cnt["tm"][ts] += 1
            rec[("tmdma", Cn)] = (ts, 16 * cnt["tm"][ts])
            tmch[pn].append(Cn)

    srow_free = {0: None, 1: None}
    pend_dve = []
    pend_pe = []
    pend_act = []

    def flush_dve():
        ops["dve"].extend(pend_dve)
        pend_dve.clear()

    def flush_pe():
        ops["pe"].extend(pend_pe)
        pend_pe.clear()

    def flush_act():
        ops["act"].extend(pend_act)
        pend_act.clear()

    def plan_av(p_, h, defer_norm=False):
        flush_pe()
        flush_act()
        C = CO[0]
        peb = C >= 6
        rec[("ph", C)] = (p_, h)
        rec[("peb", C)] = peb
        slot = ps_at[C % 3]
        if C == 0:
            wait("pe", s_dve, rec[("copy", 7)])   # va tb0-3 ready
        if C == 8:
            wait("pe", s_dve, rec[("copy", 19)])  # va tb4-7 ready
        if C >= 3:
            wait("pe", s_dr, rec[("norm", C - 3)])
        njb = 4 if p_ == 0 else 8
        nsl = 4
        lgstart = rec[("lgend", (p_, h))] - njb
        for k, jb in enumerate(range(njb)):
            st, sp_ = k == 0, k == njb - 1
            wait("pe", s_act, lgstart + jb + 1)
            fn = (lambda h=h, nsl=nsl, jb=jb, p_=p_, slot=slot, st=st, sp_=sp_: lambda e: e.matmul(
                slot[:, :], va_sb[:, jb, h, 0:65], pt_sb[:, h % nsl, jb, p_ * 512:(p_ + 1) * 512],
                start=st, stop=sp_))()
            op("pe", fn, (s_pe, 1) if sp_ else None)
        cnt["pe"] += 1
        rec[("av", C)] = cnt["pe"]
        rec[("avdone", (p_, h))] = cnt["pe"]

        cs = C % 2
        if peb:
            # DVE: reciprocal of the whole sum row straight out of PSUM
            wait("dve", s_pe, rec[("av", C)])
            if srow_free[cs] is not None:
                wait("dve", srow_free[cs][0], srow_free[cs][1])
            op("dve", (lambda cs=cs, slot=slot: lambda e: e.reciprocal(
                srow[64:65, cs, 0:256], slot[64:65, 0:256]))(), (s_rp, 1))
            cnt["rp"] += 1
            rec[("rcpA", C)] = cnt["rp"]
            op("dve", (lambda cs=cs, slot=slot: lambda e: e.reciprocal(
                srow[64:65, cs, 256:512], slot[64:65, 256:512]))(), (s_rp, 1))
            cnt["rp"] += 1
            rec[("rcp", C)] = cnt["rp"]
            flush_dve()  # prior norms queue behind the critical-path recips

            # PE: broadcast 1/s across 64 partitions via ones-stationary
            # matmul; deferred in the PE queue so the row-reciprocal latency
            # hides under the next logits/AV work
            pend_pe.append(("wait", s_rp, rec[("rcpA", C)]))
            if C == 6:
                pend_pe.append(("wait", s_dve, rec[("copy", 22)]))
            elif C == 7:
                pend_pe.append(("wait", s_dve, rec[("copy", 23)]))
            elif C == 8:
                pend_pe.append(("wait", s_dve, rec[("stage", 6)]))
            elif C == 9:
                pend_pe.append(("wait", s_dve, rec[("stage", 7)]))
            else:
                pend_pe.append(("wait", s_bc, rec[("bccp", C - 2)]))
            fn = (lambda C=C, cs=cs: lambda e: e.matmul(
                ps_mm[C % 2][0:64, 0:256], on_sb[64:65, 0:64], srow[64:65, cs, 0:256],
                start=True, stop=True))()
            pend_pe.append(("op", fn, (s_bm, 1)))
            cnt["bm"] += 1
            pend_pe.append(("wait", s_rp, rec[("rcp", C)]))
            fn = (lambda C=C, cs=cs: lambda e: e.matmul(
                ps_mm[C % 2][0:64, 256:512], on_sb[64:65, 0:64], srow[64:65, cs, 256:512],
                start=True, stop=True))()
            pend_pe.append(("op", fn, (s_bm, 1)))
            cnt["bm"] += 1
            rec[("bcmm", C)] = cnt["bm"]
            srow_free[cs] = (s_bm, cnt["bm"])

            # Copy the broadcast out of PSUM (DVE can't read 2 PSUM inputs).
            # Mid-phase chunks: on ACT, deferred past the exps PE still needs.
            # Phase-end chunks: ph0 on DVE (act busy with next-phase exps),
            # ph1 on ACT directly (act idle there).
            fn = (lambda C=C, cs=cs: lambda e: e.copy(
                rbc[cs][:, :], ps_mm[C % 2][0:64, 0:512]))()
            if C in (6, 7):
                fn = (lambda C=C, cs=cs: lambda e: e.tensor_copy(
                    rbc[cs][:, :], ps_mm[C % 2][0:64, 0:512]))()
                wait("dve", s_bm, rec[("bcmm", C)])
                if C >= 2 and ("norm", C - 2) in rec:
                    wait("dve", s_dr, rec[("norm", C - 2)])
                op("dve", fn, (s_bc, 1))
            elif C >= 14:
                wait("act", s_bm, rec[("bcmm", C)])
                if ("norm", C - 2) in rec:
                    wait("act", s_dr, rec[("norm", C - 2)])
                op("act", fn, (s_bc, 1))
            else:
                pend_act.append(("wait", s_bm, rec[("bcmm", C)]))
                if ("norm", C - 2) in rec:
                    pend_act.append(("wait", s_dr, rec[("norm", C - 2)]))
                pend_act.append(("op", fn, (s_bc, 1)))
            cnt["bc"] += 1
            rec[("bccp", C)] = cnt["bc"]
        else:
            # DVE: copy the sum row out of PSUM
            wait("dve", s_pe, rec[("av", C)])
            if srow_free[cs] is not None:
                wait("dve", srow_free[cs][0], srow_free[cs][1])
            op("dve", (lambda cs=cs, slot=slot: lambda e: e.tensor_copy(
                srow[64:65, cs, :], slot[64:65, :]))(), (s_dve, 1))
            cnt["dve"] += 1
            rec[("rowcp", C)] = cnt["dve"]
            flush_dve()

            # GP: columnarize row -> sP[128, 4] (sbuf->sbuf)
            wait("gp", s_dve, rec[("rowcp", C)])
            wait("gp", s_r2[cs], 16 * cnt["r2"][cs])  # sP slot free
            op("gp", (lambda cs=cs: lambda e: e.dma_start(
                out=sP[:, cs, :], in_=srow[64:65, cs, :]))(), (s_rd[cs], 16))
            cnt["rd"][cs] += 1
            srow_free[cs] = (s_rd[cs], 16 * cnt["rd"][cs])

            # DVE: reciprocal on [128, 4] (free size 4: fast), deferred in the
            # queue so its col-DMA completion wait never blocks QKV copies
            pend_dve.append(("wait", s_rd[cs], 16 * cnt["rd"][cs]))
            pend_dve.append(("op", (lambda cs=cs: lambda e: e.reciprocal(
                sP[:, cs, :], sP[:, cs, :]))(), (s_rp, 1)))
            cnt["rp"] += 1
            rec[("rcp", C)] = cnt["rp"]

            # GP: sP -> r2_dram -> broadcast to rbc[64, 512]
            wait("gp", s_rp, rec[("rcp", C)])
            wait("gp", s_rbc[cs], 16 * cnt["rbc"][cs])  # r2_dram slot free
            op("gp", (lambda cs=cs: lambda e: e.dma_start(
                out=r2_dram[cs, :, :], in_=sP[:, cs, :]))(), (s_r2[cs], 16))
            cnt["r2"][cs] += 1
            wait("gp", s_r2[cs], 16 * cnt["r2"][cs])
            if C >= 2:
                wait("gp", s_dr, rec[("norm", C - 2)])  # rbc slot free
            op("gp", (lambda cs=cs: lambda e: e.dma_start(
                out=rbc[cs][:, :],
                in_=r2_dram[cs, None, :, :].broadcast_to([64, 128, 4])))(), (s_rbc[cs], 16))
            cnt["rbc"][cs] += 1
            rec[("bcv", C)] = 16 * cnt["rbc"][cs]

        if h % 2 == 1:
            rec[("oidx", C)] = CO[1]
            CO[1] += 1
        if C >= 1 and not defer_norm:
            plan_norm(C - 1)
        CO[0] += 1

    # --- output projection quarter q + RS ---
    OG = [0]

    def plan_oproj_quarter(q):
        flush_dve()
        flush_pe()
        flush_act()
        p_ = 0 if q < 2 else 1
        groups = [(tb, mc) for tb in Q_TBS[q] for mc in range(2)]
        for j, (tb, mc) in enumerate(groups):
            og = OG[0]
            slot = ps_mm[og % 2]
            if og == 0:
                wait("pe", s_wo, 16)
                wait("pe", s_dve, rec[("copy", 23)])  # ps_mm free of QKV
            if j == 0 and q in (0, 2):
                # all norms of phase p_ done (incl. odd-head tmp DMAs)
                wait("pe", s_dr, rec[("norm", 8 * p_ + 7)])
                for Cn in tmch[p_]:
                    ts, tv = rec[("tmdma", Cn)]
                    wait("pe", s_tm[ts], tv)
            if og >= 2:
                wait("pe", s_dve, rec[("stage", og - 2)])
            for nb in range(4):
                st, sp_ = nb == 0, nb == 3
                fn = (lambda nb=nb, tb=tb, mc=mc, slot=slot, st=st, sp_=sp_: lambda e: e.matmul(
                    slot[:, :], at_sb[:, nb, tb * 128:(tb + 1) * 128], wo_sb[:, nb, mc * 512:(mc + 1) * 512],
                    start=st, stop=sp_))()
                op("pe", fn, (s_pe, 1) if sp_ else None)
            cnt["pe"] += 1
            rec[("op", og)] = cnt["pe"]

            wait("dve", s_pe, rec[("op", og)])
            if og >= 8:
                wait("dve", s_out, 16 * (og - 7))  # stg slot free
            fn = (lambda og=og, slot=slot: lambda e: e.tensor_copy(
                stg[:, og % 8, :], slot[:, :]))()
            op("dve", fn, (s_dve, 1))
            cnt["dve"] += 1
            rec[("stage", og)] = cnt["dve"]

            wait("sp", s_dve, rec[("stage", og)])
            pdst = partials[q]
            fn = (lambda j=j, og=og, pdst=pdst: lambda e: e.dma_start(
                out=pdst[(j // 2) * 128:(j // 2 + 1) * 128, (j % 2) * 512:(j % 2 + 1) * 512],
                in_=stg[:, og % 8, :]))()
            op("sp", fn, (s_out, 16))
            cnt["out"] += 1
            OG[0] += 1

        wait("gp", s_out, 16 * cnt["out"])
        op("gp", (lambda q=q: lambda e: e.collective_compute(
            "ReduceScatter", bass.mybir.AluOpType.add,
            replica_groups=[[0, 1], [2, 3], [4, 5], [6, 7]],
            ins=[partials[q].ap().opt()], outs=[reds[q].ap().opt()]))(), (s_cc, 1))

    # ---- master schedule ----
    qkv_groups = (
        [("q", 0, 0), ("k", 0, 0)],                             # A: heads 0,1 (t first half)
        [("q", 1, 0), ("k", 1, 0)],                             # B: heads 2,3
        [("v", 0), ("v", 1), ("v", 2), ("v", 3)],               # C
        [("q", 2, 0), ("k", 2, 0)],                             # D: heads 4,5
        [("q", 3, 0), ("k", 3, 0)],                             # E: heads 6,7
        [("q", 0, 1), ("k", 0, 1), ("q", 1, 1), ("k", 1, 1)],   # F: tc1 nb0/1
        [("v", 4), ("v", 5), ("v", 6), ("v", 7)],               # G
        [("q", 2, 1), ("k", 2, 1), ("q", 3, 1), ("k", 3, 1)],   # H: tc1 nb2/3
    )
    gi = [0]

    def emit_qkv(block):
        for item in qkv_groups[block]:
            g = gi[0]
            if item[0] == "q":
                if g == 0:
                    wait("pe", s_xT, 32)
                    wait("pe", s_wq, 16)
                if item[1] == 1 and item[2] == 0:
                    wait("pe", s_wq, 32)
                if item[2] == 1 and item[1] == 0:
                    wait("pe", s_xT, 48)
            elif item[0] == "k":
                if item[1] == 0 and item[2] == 0:
                    wait("pe", s_wk, 16)
                if item[1] == 1 and item[2] == 0:
                    wait("pe", s_wk, 32)
            else:
                if item[1] == 0:
                    wait("pe", s_wv, 16)
            plan_qkv(item, g)
            gi[0] += 1

    emit_qkv(0)                      # g0-1
    plan_logits(0, 0)
    plan_logits(0, 1)
    emit_qkv(1)                      # g2-3
    plan_logits(0, 2)
    plan_logits(0, 3)
    emit_qkv(2)                      # g4-7: v tb0-3
    plan_av(0, 0)
    emit_qkv(3)                      # g8-9
    plan_av(0, 1)
    plan_logits(0, 4)
    plan_av(0, 3)
    plan_logits(0, 5)
    emit_qkv(4)                      # g10-11
    plan_av(0, 2)
    plan_logits(0, 6)
    emit_qkv(5)                      # g12-15: tc1 nb0/1
    plan_av(0, 5)
    plan_logits(0, 7)
    plan_av(0, 7)                    # C5: last odd head retired early
    emit_qkv(6)                      # g16-19: v tb4-7
    emit_qkv(7)                      # g20-23: tc1 nb2/3
    plan_av(0, 4)                    # C6: PE-broadcast drain (ps_mm free)
    plan_av(0, 6, defer_norm=True)   # C7: even heads at the gate, no tm DMA
    plan_norm(6)                     # drain phase 0
    plan_norm(7)

    # phase 1: odd heads (which need the extra tmp->at DMA) run early so the
    # tail drains on even heads only
    plan_logits(1, 0)
    plan_logits(1, 1)
    plan_logits(1, 2)
    plan_logits(1, 3)
    plan_oproj_quarter(0)
    plan_oproj_quarter(1)
    plan_av(1, 1)                    # C8
    plan_logits(1, 5)
    plan_av(1, 3)                    # C9
    plan_logits(1, 7)
    plan_av(1, 2)                    # C10
    plan_logits(1, 6)
    plan_av(1, 5)                    # C11
    plan_av(1, 7)                    # C12
    plan_av(1, 0)                    # C13
    plan_logits(1, 4)
    plan_av(1, 6)                    # C14
    plan_av(1, 4, defer_norm=True)   # C15
    plan_norm(14)                    # drain phase 1
    plan_norm(15)
    plan_oproj_quarter(2)

    for q in range(3):
        wait("sp", s_cc, q + 2)  # +1 for the warm-up collective
        op("sp", (lambda q=q: lambda e: e.dma_start(
            out=outs_p[q][:, :], in_=reds[q][:, :]))(), (s_fin, 16))
    wait("gp", s_fin, 48)

    _CACHE["ops_debug"] = {k: list(v) for k, v in ops.items()}

    # ---- emit ----
    def emit(eng, lst):
        for item in lst:
            if item[0] == "wait":
                eng.wait_ge(item[1], item[2])
            else:
                inst = item[1](eng)
                if item[2] is not None:
                    inst.then_inc(item[2][0], item[2][1])

    with nc.allow_low_precision("bf16 softmax normalization broadcast"), nc.Block() as block:
        @block.sync
        def _(e):
            emit(e, ops["sp"])

        @block.tensor
        def _(e):
            emit(e, ops["pe"])

        @block.vector
        def _(e):
            emit(e, ops["dve"])

        @block.scalar
        def _(e):
            emit(e, ops["act"])

        @block.gpsimd
        def _(e):
            emit(e, ops["gp"])

    ctx.close()
    return nc


def _get_nc():
    if "nc" not in _CACHE:
        _CACHE["nc"] = _build()
    return _CACHE["nc"]


def _prep_inputs(x, Wq, Wk, Wv, Wo, bo, rel_pos_bias):
    bf = ml_dtypes.bfloat16
    in_maps = []
    p_idx = np.arange(128)[:, None]
    u_idx = np.arange(T)[None, :]
    for core in range(NCORES):
        b, g = core // 2, core % 2
        xb = np.asarray(x[b], dtype=np.float32)
        xT_h = np.ascontiguousarray(
            xb.T.reshape(8, 128, T).transpose(1, 0, 2)).astype(bf)
        wq_h = np.ascontiguousarray(
            Wq[:, g * NL:(g + 1) * NL].reshape(8, 128, NL).transpose(1, 0, 2)).astype(bf)
        wk_h = np.ascontiguousarray(
            Wk[:, g * NL:(g + 1) * NL].reshape(8, 128, NL).transpose(1, 0, 2)).astype(bf)
        wv_h = np.ascontiguousarray(
            Wv[:, g * NL:(g + 1) * NL].reshape(8, 128, NL).transpose(1, 0, 2)).astype(bf)
        wo_h = np.ascontiguousarray(
            Wo[g * NL:(g + 1) * NL, :].reshape(4, 128, D).transpose(1, 0, 2)).astype(bf)
        db = np.empty((128, HL, T), dtype=bf)
        dif = np.clip(u_idx - p_idx, 0, T - 1)
        msk = u_idx >= p_idx
        for h in range(HL):
            rev = np.asarray(rel_pos_bias[g * HL + h], dtype=np.float32)[::-1]
            db[:, h, :] = np.where(msk, rev[dif], NEG).astype(bf)
        in_maps.append({
            "xT": xT_h,
            "wq0": np.ascontiguousarray(wq_h[:, :, 0:128]),
            "wq1": np.ascontiguousarray(wq_h[:, :, 128:512]),
            "wk0": np.ascontiguousarray(wk_h[:, :, 0:128]),
            "wk1": np.ascontiguousarray(wk_h[:, :, 128:512]),
            "wv": wv_h, "wo": wo_h,
            "dbias": db, "ident": np.eye(128, dtype=bf),
        })
    return in_maps


def run_on_device(x, Wq, Wk, Wv, Wo, bo, rel_pos_bias, trace=False):
    from concourse.bass_utils import run_bass_kernel_spmd

    nc = _get_nc()
    in_maps = _prep_inputs(x, Wq, Wk, Wv, Wo, bo, rel_pos_bias)
    res = run_bass_kernel_spmd(nc, in_maps, core_ids=list(range(NCORES)), trace=trace)
    bo_f = np.asarray(bo, np.float32)
    outs = []
    for b in range(B):
        ev = res.results[2 * b]
        od = res.results[2 * b + 1]
        rows = []
        for q in range(3):
            rows.append(ev[f"out{q}"])
            rows.append(od[f"out{q}"])
        outs.append(np.concatenate(rows, axis=0))
    out = np.stack(outs).astype(np.float32) + bo_f[None, None, :]
    return out, res


def kernel(x, Wq, Wk, Wv, Wo, bo, rel_pos_bias):
    out, _ = run_on_device(x, Wq, Wk, Wv, Wo, bo, rel_pos_bias, trace=False)
    return out
